# revision 24
# baseline (speedup 1.0000x reference)
"""nn_MultiHeadAttention TRN2 kernel: 8-core tensor-parallel (2 heads/core).

Self-contained: builds and compiles the Bass/Tile SPMD program on first call,
shards the full inputs per-core on the host, runs via run_bass_kernel_spmd,
and concatenates the per-core sequence-block outputs into the full output.

Algorithm (per core, 2 heads of 16, head_dim 64, S=4096, D=1024):
  - feature-major layout: xT [D,S]; q/k projected with RoPE-permuted,
    transposed weight shards; the rotary partner permutation is chosen so the
    rotation is a 16-lane half-swap within every 32-partition block
    (one DVE stream_shuffle), with signs folded into the sin table.
  - chunked pipeline: per 512-query chunk, project q/k (PE), evacuate+RoPE
    (ACT+DVE), project v into an augmented seq-major tile with a ones column
    (softmax denominator); attention for query block Q starts as soon as its
    chunks are ready.
  - every matmul runs in the full 128x128 PE mode (no tiling-mode switches):
    scores contract K=128 against per-head key tiles whose other-head
    partitions are zeroed; PV contracts K=128 with M=65 (row 64 = ones =
    softmax denominator).
  - flash attention on transposed score tiles scoresT[k,q]: scores for a
    jt-pair land in a [128,1024] PSUM tile (3-slot rotation), causal mask
    added as a -400 triangle on diagonal tiles (DVE), exp on ScalarE over
    [128,1024] per head-pair, PV accumulates outT[65,512] per head.
  - normalize: both heads' denominator rows gather into one [2,512] tile,
    one DVE reciprocal, one 128x128 select-matmul broadcasts the reciprocals
    across partitions, two DVE multiplies; per-block staging DMA to the
    AllToAll input overlaps later blocks.
  - AllToAll re-shards from head-split to sequence-split; final projection
    against full Wo.T; each core emits out[512, 1024] f32.
"""

from contextlib import ExitStack

import numpy as np
import ml_dtypes

import concourse.tile as tile
from concourse import bacc, mybir
from concourse.bass_utils import run_bass_kernel_spmd

F32 = mybir.dt.float32
BF16 = mybir.dt.bfloat16

S = 4096
D = 1024
HD = 64
N_CORES = 8
KT = 128
BQ = 512
NFT = D // 128          # 8 feature tiles
NKT = S // KT           # 32 key tiles
NQB = S // BQ           # 8 query blocks
CHUNK = S // N_CORES    # 512

SHUF_HALF = list(range(16, 32)) + list(range(16))


def _build():
    nc = bacc.Bacc("TRN2", target_bir_lowering=False, debug=False, num_devices=N_CORES)

    xT = nc.dram_tensor("xT", [D, S], BF16, kind="ExternalInput")
    wq = nc.dram_tensor("wq", [D, 128], BF16, kind="ExternalInput")
    wk = nc.dram_tensor("wk", [D, 128], BF16, kind="ExternalInput")
    wv = nc.dram_tensor("wv", [D, 128], BF16, kind="ExternalInput")
    wo = nc.dram_tensor("wo", [D, D], BF16, kind="ExternalInput")
    cosP = nc.dram_tensor("cosP", [128, S], BF16, kind="ExternalInput")
    sinN = nc.dram_tensor("sinN", [128, S], BF16, kind="ExternalInput")
    lu = nc.dram_tensor("lu", [128, 128], BF16, kind="ExternalInput")
    out = nc.dram_tensor("out", [CHUNK, D], F32, kind="ExternalOutput")

    a2a_in = nc.dram_tensor("a2a_in", [N_CORES * 128, CHUNK], BF16)
    a2a_out = nc.dram_tensor("a2a_out", [N_CORES * 128, CHUNK], BF16)

    with tile.TileContext(nc) as tc, ExitStack() as ctx:
        sb = ctx.enter_context(tc.tile_pool(name="sb", bufs=1))
        # PSUM: 3 x [128,1024] f32 (6 banks) + 2 x [65,512] (2 banks) = 8 banks
        psc = ctx.enter_context(tc.tile_pool(name="psc", bufs=3, space="PSUM"))
        ppv = ctx.enter_context(tc.tile_pool(name="ppv", bufs=2, space="PSUM"))

        xt_s = [sb.tile([128, S], BF16, tag=f"xt{t}", name=f"xt{t}") for t in range(NFT)]
        wq_s = sb.tile([128, NFT * 128], BF16, tag="wq", name="wq_s")
        wk_s = sb.tile([128, NFT * 128], BF16, tag="wk", name="wk_s")
        wv_s = sb.tile([128, NFT * 128], BF16, tag="wv", name="wv_s")
        wo_s = [sb.tile([128, D], BF16, tag=f"wo{t}", name=f"wo_s{t}") for t in range(NFT)]
        cos_s = sb.tile([128, S], BF16, tag="cos", name="cos_s")
        sin_s = sb.tile([128, S], BF16, tag="sin", name="sin_s")
        lu_s = sb.tile([128, 128], BF16, tag="lu", name="lu_s")
        qTc = [sb.tile([128, BQ], BF16, tag=f"qT{c}", name=f"qTc{c}") for c in range(NQB)]
        # per-head key tiles: other head's 64 partitions stay zero so score
        # matmuls can contract the full K=128 (keeps PE in 128x128 mode)
        kTc = [
            [sb.tile([128, BQ], BF16, tag=f"kT{h}_{c}", name=f"kTc{h}_{c}") for c in range(NQB)]
            for h in (0, 1)
        ]
        v_aug = sb.tile([128, NKT * 256], BF16, tag="vaug", name="v_aug")
        aT = [sb.tile([128, CHUNK], BF16, tag=f"aT{t}", name=f"aT{t}") for t in range(NFT)]

        for t in range(NFT):
            nc.sync.dma_start(wq_s[:, 128 * t : 128 * (t + 1)], wq[128 * t : 128 * (t + 1), :])
            nc.sync.dma_start(wk_s[:, 128 * t : 128 * (t + 1)], wk[128 * t : 128 * (t + 1), :])
        for t in range(NFT):
            nc.sync.dma_start(xt_s[t][:, 0 : S // 2], xT[128 * t : 128 * (t + 1), 0 : S // 2])
        nc.sync.dma_start(cos_s[:], cosP[:, :])
        nc.sync.dma_start(sin_s[:], sinN[:, :])
        nc.sync.dma_start(lu_s[:], lu[:, :])
        for t in range(NFT):
            nc.sync.dma_start(wv_s[:, 128 * t : 128 * (t + 1)], wv[128 * t : 128 * (t + 1), :])
        for t in range(NFT):
            nc.gpsimd.dma_start(xt_s[t][:, S // 2 : S], xT[128 * t : 128 * (t + 1), S // 2 : S])
        for t in range(NFT):
            nc.gpsimd.dma_start(wo_s[t][:], wo[128 * t : 128 * (t + 1), :])

        for c in range(NQB):
            nc.vector.memset(kTc[0][c][64:128, :], 0.0)
            nc.vector.memset(kTc[1][c][0:64, :], 0.0)

        # sel: broadcast-matmul weights — output partition m takes rb row 0
        # (m<64) or row 64 (m>=64); K=128 x M=128 keeps the PE mode uniform
        sel = sb.tile([128, 128], BF16, tag="sel", name="sel")
        nc.vector.memset(sel[:], 0.0)
        nc.vector.memset(sel[0:1, 0:64], 1.0)
        nc.vector.memset(sel[64:65, 64:128], 1.0)
        # rb rows 0/64 receive the per-head denominators; all other rows stay
        # 1.0 so the full-tile in-place reciprocal is NaN/Inf-free
        rb = sb.tile([128, BQ], F32, tag="rb", name="rb")
        nc.vector.memset(rb[:], 1.0)
        rb16 = sb.tile([128, BQ], BF16, tag="rb16", name="rb16")
        nc.vector.memset(rb16[:], 1.0)
        # rb rows 0/64 receive the per-head denominators; the rest stays 1.0
        # so the full-tile reciprocal never divides by zero
        rb = sb.tile([128, BQ], F32, tag="rb", name="rb")
        nc.vector.memset(rb[:], 1.0)
        rb16 = sb.tile([128, BQ], BF16, tag="rb16", name="rb16")
        nc.vector.memset(rb16[:], 1.0)

        # v_aug: per key tile, two 128-wide stationary blocks (one per head):
        # [64 v dims | ones | 63 zero pad]; 128 columns keep LDWEIGHTS on the
        # fast-weight-load path
        nc.vector.memset(v_aug[:], 0.0)
        for st in range(NKT):
            nc.vector.memset(v_aug[:, 256 * st + 64 : 256 * st + 65], 1.0)
            nc.vector.memset(v_aug[:, 256 * st + 192 : 256 * st + 193], 1.0)

        def _piece_qk(nb, is_q):
            cs = slice(BQ * nb, BQ * (nb + 1))
            w_s = wq_s if is_q else wk_s
            p = psc.tile([128, BQ], F32, tag="sc", name="p_qk")
            for t in range(NFT):
                nc.tensor.matmul(
                    p[:],
                    w_s[:, 128 * t : 128 * (t + 1)],
                    xt_s[t][:, cs],
                    start=(t == 0),
                    stop=(t == NFT - 1),
                )
            a = sb.tile([128, BQ], BF16, tag="ropeA", name="rope_a", bufs=2)
            nc.scalar.copy(a[:], p[:])
            b = sb.tile([128, BQ], BF16, tag="ropeB", name="rope_b", bufs=2)
            nc.vector.stream_shuffle(b[:], a[:], SHUF_HALF)
            t1 = sb.tile([128, BQ], BF16, tag="ropeT", name="rope_t", bufs=2)
            nc.vector.tensor_mul(t1[:], a[:], cos_s[:, cs])
            nc.vector.tensor_mul(b[:], b[:], sin_s[:, cs])
            if is_q:
                nc.vector.tensor_add(qTc[nb][:], t1[:], b[:])
            else:
                nc.vector.tensor_add(kTc[0][nb][0:64, :], t1[0:64, :], b[0:64, :])
                nc.vector.tensor_add(kTc[1][nb][64:128, :], t1[64:128, :], b[64:128, :])

        def _piece_v(st):
            pv = psc.tile([128, KT], F32, tag="sc", name="p_v")
            for t in range(NFT):
                nc.tensor.matmul(
                    pv[:],
                    xt_s[t][:, KT * st : KT * (st + 1)],
                    wv_s[:, 128 * t : 128 * (t + 1)],
                    start=(t == 0),
                    stop=(t == NFT - 1),
                )
            nc.vector.tensor_copy(v_aug[:, 256 * st : 256 * st + 64], pv[:, 0:64])
            nc.vector.tensor_copy(v_aug[:, 256 * st + 128 : 256 * st + 192], pv[:, 64:128])

        def phase_a_pieces(nb):
            """Projection work for chunk nb as drip-schedulable closures."""
            return (
                [(nb, lambda nb=nb: _piece_qk(nb, True)), (nb, lambda nb=nb: _piece_qk(nb, False))]
                + [(nb, lambda st=st: _piece_v(st)) for st in range(4 * nb, 4 * nb + 4)]
            )

        def phase_a(nb):
            for _, piece in phase_a_pieces(nb):
                piece()

        def phase_b(Q, hooks_prev=(), aqueue=None):
            done_prev = []
            """Attention for query block Q (both heads, all 128x128-mode MMs).
            Returns a closure that emits the deferred normalize+stage for this
            block; the caller runs it once the next block's scores are queued
            so the broadcast matmul never head-of-line-blocks the PE."""
            q0 = BQ * Q
            n_jt = (q0 + BQ) // KT
            n_pair = n_jt // 2
            outT = [
                ppv.tile([128, BQ], F32, tag="pv", name=f"outT{Q}_h{h}") for h in (0, 1)
            ]
            for pr in range(n_pair):
                jts = (2 * pr, 2 * pr + 1)
                sch = [
                    psc.tile([128, 2 * BQ], F32, tag="sc", name=f"sc_h{h}") for h in (0, 1)
                ]
                for j, jt in enumerate(jts):
                    for h in (0, 1):
                        nc.tensor.matmul(
                            sch[h][:, BQ * j : BQ * (j + 1)],
                            kTc[h][jt // 4][:, 128 * (jt % 4) : 128 * (jt % 4 + 1)],
                            qTc[Q][:, :],
                            start=True,
                            stop=True,
                        )
                for j, jt in enumerate(jts):
                    if KT * jt >= q0:
                        off = KT * jt - q0
                        for h in (0, 1):
                            nc.vector.tensor_add(
                                sch[h][:, BQ * j + off : BQ * j + off + 128],
                                sch[h][:, BQ * j + off : BQ * j + off + 128],
                                lu_s[:],
                            )
                due = [h for slot, h in hooks_prev if pr >= slot and h not in done_prev]
                if due:
                    due[0]()
                    done_prev.append(due[0])
                elif aqueue:
                    _, piece = aqueue.pop(0)
                    piece()
                ex = []
                for h in (0, 1):
                    e = sb.tile([128, 2 * BQ], BF16, tag="expT", name=f"expT_h{h}", bufs=4)
                    nc.scalar.activation(
                        e[:, :],
                        sch[h][:, :],
                        mybir.ActivationFunctionType.Exp,
                        scale=0.125,
                    )
                    ex.append(e)
                first = pr == 0
                last = pr == n_pair - 1
                for j, jt in enumerate(jts):
                    # columns left of the diagonal block are fully masked but
                    # carry garbage exp values -- PV must skip them
                    trim = max(0, KT * jt - q0)
                    for h in (0, 1):
                        nc.tensor.matmul(
                            outT[h][:, trim:BQ],
                            v_aug[:, 256 * jt + 128 * h : 256 * jt + 128 * (h + 1)],
                            ex[h][:, BQ * j + trim : BQ * (j + 1)],
                            start=(first and j == 0),
                            stop=(last and j == 1),
                        )
            # evacuate outT to SBUF right away so the PSUM banks free for the
            # next block's PV; the rest of the normalize is deferred
            ov0 = sb.tile([65, BQ], F32, tag="ov0", name="ov0", bufs=2)
            # head-1 values live at partitions 64-127 (same base as their bc
            # rows -- SBUF/SBUF ops need equal input bases); row 0 holds den1
            ov1 = sb.tile([128, BQ], F32, tag="ov1", name="ov1", bufs=2)

            def evac():
                nc.vector.tensor_copy(ov0[:, :], outT[0][0:65, :])
                nc.vector.tensor_copy(ov1[64:128, :], outT[1][0:64, :])
                nc.vector.tensor_copy(ov1[0:1, :], outT[1][64:65, :])

            def recip():
                nc.vector.tensor_copy(rb[0:1, :], ov0[64:65, :])
                nc.vector.tensor_copy(rb[64:65, :], ov1[0:1, :])
                with nc.allow_low_precision(reason="softmax denominators are O(1-500); bf16 recip adds <0.5% scale noise"):
                    nc.vector.reciprocal(rb16[:, :], rb[:, :])

            def finish():
                bcp = psc.tile([128, BQ], F32, tag="sc", name="bcp")
                nc.tensor.matmul(bcp[:], sel[:], rb16[:, :], start=True, stop=True)
                bc = sb.tile([128, BQ], F32, tag="bc", name="bc", bufs=2)
                nc.vector.tensor_copy(bc[:], bcp[:])
                attc = sb.tile([128, BQ], BF16, tag="attc", name="attc", bufs=2)
                nc.vector.tensor_mul(attc[0:64, :], ov0[0:64, :], bc[0:64, :])
                nc.vector.tensor_mul(attc[64:128, :], ov1[64:128, :], bc[64:128, :])
                nc.sync.dma_start(a2a_in[128 * Q : 128 * (Q + 1), :], attc[:])

            return evac, recip, finish

        phase_a(0)
        phase_a(1)
        hooks = ()
        aqueue = []
        for Q in range(NQB):
            if Q + 2 < NQB:
                aqueue.extend(phase_a_pieces(Q + 2))
            # anything chunk Q still queued must land before its attention
            while aqueue and aqueue[0][0] <= Q:
                aqueue.pop(0)[1]()
            ev, rc, fin = phase_b(Q, hooks, aqueue)
            hooks = ((0, ev), (1, rc), (3, fin))
        ev()
        rc()
        fin()

        nc.gpsimd.collective_compute(
            "AllToAll",
            mybir.AluOpType.bypass,
            replica_groups=[list(range(N_CORES))],
            ins=[a2a_in.ap().opt()],
            outs=[a2a_out.ap().opt()],
        )
        for t in range(NFT):
            nc.sync.dma_start(aT[t][:], a2a_out[128 * t : 128 * (t + 1), :])

        for it in range(CHUNK // 128):
            for oh in range(D // 512):
                p = psc.tile([128, 512], F32, tag="sc", name="p_o")
                for t in range(NFT):
                    nc.tensor.matmul(
                        p[:],
                        aT[t][:, 128 * it : 128 * (it + 1)],
                        wo_s[t][:, 512 * oh : 512 * (oh + 1)],
                        start=(t == 0),
                        stop=(t == NFT - 1),
                    )
                ot = sb.tile([128, 512], F32, tag="oflush", name="ot", bufs=2)
                nc.scalar.copy(ot[:], p[:])
                nc.sync.dma_start(
                    out[128 * it : 128 * (it + 1), 512 * oh : 512 * (oh + 1)], ot[:]
                )

    nc.compile()
    return nc


def _host_prep(x, Wq, Wk, Wv, Wo):
    bf = ml_dtypes.bfloat16
    # rotary partner permutation: within each head, arrange the 64 dims so a
    # rotation partner is 16 partitions away inside the same 32-block:
    # [e0..e15, o0..o15, e16..e31, o16..o31]
    perm = np.concatenate(
        [
            np.arange(0, 32, 2),
            np.arange(1, 32, 2),
            np.arange(32, 64, 2),
            np.arange(33, 64, 2),
        ]
    )
    pp = np.arange(64)
    pair_i = np.where(pp < 16, pp, np.where(pp < 48, pp - 16, pp - 32))
    sign = np.where((pp % 32) < 16, -1.0, 1.0).astype(np.float32)

    inv_freq = 1.0 / (10000.0 ** (np.arange(0, HD, 2, dtype=np.float32) / HD))
    fr = np.outer(np.arange(S, dtype=np.float32), inv_freq)  # [S, 32]
    cosA = np.cos(fr).T  # [32, S]
    sinA = np.sin(fr).T
    cos64 = cosA[pair_i]
    sin64 = sinA[pair_i] * sign[:, None]
    cosP = np.tile(cos64, (2, 1)).astype(bf)
    sinN = np.tile(sin64, (2, 1)).astype(bf)
    lu = np.tril(np.full((128, 128), -400.0, np.float32), k=-1).astype(bf)

    xT = np.ascontiguousarray(x.reshape(S, D).T).astype(bf)
    woT = np.ascontiguousarray(np.asarray(Wo, np.float32).T).astype(bf)

    in_maps = []
    for c in range(N_CORES):
        rows = np.concatenate([128 * c + 64 * h + perm for h in range(2)])
        in_maps.append(
            {
                "xT": xT,
                "wq": np.ascontiguousarray(np.asarray(Wq, np.float32)[rows].T).astype(bf),
                "wk": np.ascontiguousarray(np.asarray(Wk, np.float32)[rows].T).astype(bf),
                "wv": np.ascontiguousarray(
                    np.asarray(Wv, np.float32)[128 * c : 128 * (c + 1)].T
                ).astype(bf),
                "wo": woT,
                "cosP": cosP,
                "sinN": sinN,
                "lu": lu,
            }
        )
    return in_maps


_NC_CACHE = None


def _assemble(results):
    full = np.concatenate([results[c]["out"] for c in range(N_CORES)], axis=0)
    return full.reshape(1, S, D).astype(np.float32)


def kernel(x, Wq, Wk, Wv, Wo):
    global _NC_CACHE
    if _NC_CACHE is None:
        _NC_CACHE = _build()
    nc = _NC_CACHE
    in_maps = _host_prep(
        np.asarray(x, np.float32),
        np.asarray(Wq, np.float32),
        np.asarray(Wk, np.float32),
        np.asarray(Wv, np.float32),
        np.asarray(Wo, np.float32),
    )
    res = run_bass_kernel_spmd(nc, in_maps, core_ids=list(range(N_CORES)))
    return _assemble(res.results)
